# revision 9
# baseline (speedup 1.0000x reference)
"""Supervised-contrastive loss on 8 Trainium2 NeuronCores — symmetric version.

Math (reference):
    z = x / max(||x||, 1e-8)                  row-normalize
    sim = (z @ z.T) / TEMP                    [N, N]
    per-anchor: pos-mean over same-class (excl. self) and logsumexp over
    j != i, then per-class mean, then mean over classes.

sim is symmetric, so only half of it is computed.  The matrix is tiled
into a 16x16 grid of [512, 512] blocks; block (a, b=(a+k) mod 16) for
k=0..8 covers every unordered pair exactly once (k=8 only for a<8).
Core c owns row-strips a=c (9 blocks) and a=c+8 (8 blocks) — 17 blocks
per core, identical program on every core (SPMD), only the data differs.
Each core's z8 columns are rolled by 512*c so its column ring is
memory-contiguous: strip 1 reads local cols [0, 4608), strip 2 reads
[4096, 8192).

Per block the core computes sim rows (fp8 DoubleRow matmuls, fp32 PSUM),
then ScalarE evaluates e = exp(10*sim - 2) writing fp8 to SBUF with the
row-sums riding accum_out.  For off-diagonal blocks the column-sums
(= the transposed rows' exp sums) are recovered with a ones-stationary
DoubleRow matmul over the fp8 exp values — 1/24 of the main matmul cost.
Class-segment sums tm = A @ W.T (W[c] = class-summed z) run at the END
so the tensor engine has scalar-independent work under the final ACT
drain.  Host assembles es[i] from row + column partials, subtracts the
exact diagonal exp(10*||z8_i||^2), and finishes the scalar loss.

Startup is DMA-trigger-bound (~620ns per trigger after a ~7us fixed
engine-init preamble), so the first chunk is only 512 wide and the DMA
order is: a8[m0] (96KB), the three 512-col z8 tiles it needs, then the
rest in consumption order.

Hardware pitfalls baked in (from the baseline's crash log): DMAs only
from nc.sync, one matmul accumulation group per PSUM bank, DoubleRow
outputs only with all 128 partitions, one EXP per psum tile.
"""

import numpy as np
import ml_dtypes

N = 8192           # anchors
D = 768            # feature dim
NOP = 64           # number of classes
CORES = 8
G = 512            # row/col block size
NGRP = N // G      # 16 groups
KT8 = D // 256     # 3 double-row contraction tiles
GW = 2048          # z8 DMA chunk width
NG = N // GW       # 4 DMA groups
TEMP_INV = 10.0
EPS = 1e-8
EBIAS = -2.0       # exp(10*sim + EBIAS); undone on host

# (strip, chunk) list: (col base, width); strip1 = blocks k=0..8 at local
# cols [0, 4608), strip2 = k=0..7 at [4096, 8192).
CHUNKS = [
    (0, 0, 512), (0, 512, 1536), (0, 2048, 1536), (0, 3584, 1024),
    (1, 4096, 1536), (1, 5632, 1536), (1, 7168, 1024),
]
NSLOT = 4 * len(CHUNKS)   # 28 pacc slots: chunk_lin*4 + m
NCS = 15                  # exported column-sum vectors per core

FP8 = ml_dtypes.float8_e4m3

_CACHE = {}
LAST_RESULT = None  # BassKernelResults of the most recent run (for profiling)


def _cs_blocks():
    """Host/device-shared program-order list of colsum exports.

    Yields (strip s, k distance) for every off-diagonal block.
    """
    out = []
    for s, base, width in CHUNKS:
        sbase = 0 if s == 0 else 4096
        for j in range(width // G):
            k = (base - sbase) // G + j
            if k == 0:
                continue  # diagonal block: no colsum
            out.append((s, k))
    return out


def _build_nc():
    from concourse import bacc
    import concourse.mybir as mybir
    import concourse.tile as tile

    f8 = mybir.dt.float8e4
    f32 = mybir.dt.float32
    Exp = mybir.ActivationFunctionType.Exp
    DR = mybir.MatmulPerfMode.DoubleRow

    nc = bacc.Bacc(
        "TRN2", target_bir_lowering=False, debug=False, enable_asserts=False
    )
    z8 = nc.dram_tensor("z8", [128, KT8, NG, 2, GW], f8, kind="ExternalInput").ap()
    a8 = nc.dram_tensor("a8", [128, 8, KT8, 2, 128], f8, kind="ExternalInput").ap()
    w8 = nc.dram_tensor("w8", [128, KT8, 2, NOP], f8, kind="ExternalInput").ap()
    on8 = nc.dram_tensor("on8", [128, 2, 128], f8, kind="ExternalInput").ap()
    pout = nc.dram_tensor("pout", [128, NSLOT], f32, kind="ExternalOutput").ap()
    cs = nc.dram_tensor("cs", [NCS, G], f32, kind="ExternalOutput").ap()
    tm = nc.dram_tensor("tm", [128, 8, NOP], f32, kind="ExternalOutput").ap()

    with tile.TileContext(nc) as tc:
        with (
            tc.tile_pool(name="zin", bufs=KT8 * (NG + 1)) as zin,
            tc.tile_pool(name="expp", bufs=4) as expp,
            tc.tile_pool(name="csb", bufs=2) as csb,
            tc.tile_pool(name="singles", bufs=1) as singles,
        ):
            # ---- input DMAs, in consumption order ----
            a8_sb = singles.tile([128, 8, KT8, 2, 128], f8)
            nc.sync.dma_start(out=a8_sb[:, 0], in_=a8[:, 0])

            z8_sb = {}  # (g, kk) -> (tile, col offset within tile)

            def dma_z8(g, kk, lo, hi):
                z8_t = zin.tile([128, 2, hi - lo], f8, name="z8_t", tag="z8_t")
                nc.sync.dma_start(out=z8_t, in_=z8[:, kk, g, :, lo:hi])
                return z8_t

            # first 512-col block of group 0, then the rest of group 0
            z8_first = [dma_z8(0, kk, 0, G) for kk in range(KT8)]
            nc.sync.dma_start(out=a8_sb[:, 1:4], in_=a8[:, 1:4])
            nc.sync.dma_start(out=a8_sb[:, 4:8], in_=a8[:, 4:8])
            on8_sb = singles.tile([128, 2, 128], f8)
            nc.sync.dma_start(out=on8_sb, in_=on8)
            z8_rest = [dma_z8(0, kk, G, GW) for kk in range(KT8)]
            w8_sb = singles.tile([128, KT8, 2, NOP], f8)
            nc.sync.dma_start(out=w8_sb, in_=w8)
            for g in range(1, NG):
                for kk in range(KT8):
                    z8_sb[(g, kk)] = (dma_z8(g, kk, 0, GW), 0)

            def z8_slice(col, kk):
                """[128, 2, 512] moving operand for local col block."""
                g, off = divmod(col, GW)
                if g == 0:
                    if off == 0:
                        return z8_first[kk]
                    return z8_rest[kk][:, :, off - G:off - G + G]
                t, _ = z8_sb[(g, kk)]
                return t[:, :, off:off + G]

            pacc = singles.tile([128, NSLOT], f32)
            ebias_sb = singles.tile([128, 1], f32)
            nc.vector.memset(ebias_sb, EBIAS)

            ps_pool = tc.alloc_tile_pool(name="ps", bufs=2, space="PSUM")
            cs_pool = tc.alloc_tile_pool(name="csp", bufs=2, space="PSUM")

            # ---- main symmetric slab ----
            cs_idx = 0
            for ch, (s, base, width) in enumerate(CHUNKS):
                nblk = width // G
                sbase = 0 if s == 0 else 4096
                exp_t = [
                    expp.tile([128, 2, width], f8, name="exp_t", tag="exp_t")
                    for _ in range(2)
                ]
                for m in range(4):
                    ps_t = ps_pool.tile([128, width], f32, name="ps_t", tag="ps_t")
                    for kk in range(KT8):
                        lhsT = a8_sb[:, s * 4 + m, kk, :, :]
                        for j in range(nblk):
                            nc.tensor.matmul(
                                ps_t[:, j * G:(j + 1) * G],
                                lhsT,
                                z8_slice(base + j * G, kk),
                                start=(kk == 0),
                                stop=(kk == KT8 - 1),
                                perf_mode=DR,
                            )
                    pair, plane = divmod(m, 2)
                    nc.scalar.activation(
                        out=exp_t[pair][:, plane, :],
                        in_=ps_t,
                        func=Exp,
                        scale=TEMP_INV,
                        bias=ebias_sb,
                        accum_out=pacc[:, ch * 4 + m:ch * 4 + m + 1],
                    )
                # column sums of the off-diagonal blocks in this chunk
                for j in range(nblk):
                    if base + j * G == sbase:
                        continue  # diagonal block
                    csp = cs_pool.tile([128, G], f32, name="cs_t", tag="cs_t")
                    for pair in range(2):
                        nc.tensor.matmul(
                            csp,
                            on8_sb,
                            exp_t[pair][:, :, j * G:(j + 1) * G],
                            start=(pair == 0),
                            stop=(pair == 1),
                            perf_mode=DR,
                        )
                    cst = csb.tile([128, G], f32, name="cs_sb", tag="cs_sb")
                    nc.vector.tensor_copy(cst, csp)
                    nc.sync.dma_start(
                        out=cs[cs_idx:cs_idx + 1, :], in_=cst[0:1, :]
                    )
                    cs_idx += 1
            assert cs_idx == NCS

            nc.sync.dma_start(out=pout, in_=pacc)

            # ---- tm = A @ W.T at the end: tensor work under the ACT drain
            tm_sb = singles.tile([128, 8, NOP], f32)
            for m8 in range(8):
                pst = ps_pool.tile([128, NOP], f32, name="ps_t", tag="ps_t")
                for kk in range(KT8):
                    nc.tensor.matmul(
                        pst,
                        a8_sb[:, m8, kk, :, :],
                        w8_sb[:, kk, :, :],
                        start=(kk == 0),
                        stop=(kk == KT8 - 1),
                        perf_mode=DR,
                    )
                nc.vector.tensor_copy(tm_sb[:, m8, :], pst)
            nc.sync.dma_start(out=tm, in_=tm_sb)

            cs_pool.release()
            ps_pool.release()

    nc.compile()
    return nc


def _get_nc():
    if "nc" not in _CACHE:
        _CACHE["nc"] = _build_nc()
    return _CACHE["nc"]


def _pack_dr(mat_t):
    """[D, cols] -> [128, KT8, 2, cols] with d = kk*256 + i*128 + p."""
    d, cols = mat_t.shape
    return np.ascontiguousarray(
        mat_t.reshape(KT8, 2, 128, cols).transpose(2, 0, 1, 3)
    )


def kernel(x, op_ids, n_op):
    global LAST_RESULT
    from concourse.bass_utils import run_bass_kernel_spmd

    x = np.asarray(x, dtype=np.float32).reshape(-1, D)
    op_ids = np.asarray(op_ids).reshape(-1).astype(np.int64)
    n_op_i = int(np.asarray(n_op))

    # ---- host prep: normalize, quantize, class sums, diagonal ----
    norms = np.sqrt((x.astype(np.float64) ** 2).sum(axis=1))
    norms = np.maximum(norms, EPS).astype(np.float32)
    z = x / norms[:, None]

    z8 = z.astype(FP8)
    z8f = z8.astype(np.float32)

    onehot = np.zeros((N, NOP), np.float32)
    onehot[np.arange(N), op_ids] = 1.0
    W8 = (onehot.T @ z8f).astype(FP8)               # [NOP, D] fp8

    z8_packed = _pack_dr(np.ascontiguousarray(z8.T))          # [128,3,2,N]
    w8_packed = _pack_dr(np.ascontiguousarray(W8.T.astype(FP8)))
    ones8 = np.ones((128, 2, 128), FP8)
    ssq = (z8f.astype(np.float64) ** 2).sum(axis=1)  # = sim[i, i]

    in_maps = []
    for c in range(CORES):
        # rolled columns: local col t = global (512c + t) mod N
        zc = np.roll(z8_packed, -G * c, axis=3)
        zc_chunked = np.ascontiguousarray(
            zc.reshape(128, KT8, 2, NG, GW).transpose(0, 1, 3, 2, 4)
        )
        rows = np.concatenate(
            [
                z8_packed[:, :, :, c * G:(c + 1) * G],
                z8_packed[:, :, :, (c + 8) * G:(c + 9) * G],
            ],
            axis=3,
        )  # [128, 3, 2, 1024]
        a8_m = np.ascontiguousarray(
            rows.reshape(128, KT8, 2, 8, 128).transpose(0, 3, 1, 2, 4)
        )  # [128, 8, 3, 2, 128]
        in_maps.append(
            {
                "z8": zc_chunked,
                "a8": a8_m,
                "w8": w8_packed,
                "on8": ones8,
            }
        )

    nc = _get_nc()
    res = run_bass_kernel_spmd(nc, in_maps, core_ids=list(range(CORES)))
    LAST_RESULT = res

    # ---- host post: assemble es from row+col partials, finish loss ----
    cs_order = _cs_blocks()
    es_scaled = np.zeros(N, np.float64)
    tm_full = np.empty((N, NOP), np.float64)
    n_ch1 = sum(1 for s, _, _ in CHUNKS if s == 0)
    for c in range(CORES):
        pout_c = res.results[c]["pout"].astype(np.float64)  # [128, NSLOT]
        cs_c = res.results[c]["cs"].astype(np.float64)      # [NCS, G]
        tm_c = res.results[c]["tm"].astype(np.float64)      # [128, 8, NOP]
        for s in range(2):
            a = c + 8 * s
            rows = slice(a * G, (a + 1) * G)
            lo = 0 if s == 0 else n_ch1
            hi = n_ch1 if s == 0 else len(CHUNKS)
            slots = pout_c[:, lo * 4:hi * 4].reshape(128, hi - lo, 4)
            es_scaled[rows] += slots.sum(axis=1).T.reshape(G)
            tm_full[rows] = (
                tm_c[:, 4 * s:4 * s + 4, :].transpose(1, 0, 2).reshape(G, NOP)
            )
        for i, (s, k) in enumerate(cs_order):
            b = (c + 8 * s + k) % NGRP
            es_scaled[b * G:(b + 1) * G] += cs_c[i]

    es = es_scaled * np.exp(-EBIAS)
    lse = np.log(es - np.exp(TEMP_INV * ssq))
    pos_sum = TEMP_INV * (tm_full[np.arange(N), op_ids] - ssq)
    counts = np.bincount(op_ids, minlength=n_op_i).astype(np.float64)
    pos_cnt = counts[op_ids] - 1.0

    loss_i = np.where(pos_cnt > 0, -pos_sum / np.maximum(pos_cnt, 1.0) + lse, 0.0)
    cls_sum = np.bincount(op_ids, weights=loss_i, minlength=n_op_i)
    cls_loss = np.where(counts > 0, cls_sum / np.maximum(counts, 1.0), 0.0)
    return np.float32(cls_loss.mean())


# revision 13
# speedup vs baseline: 1.0121x; 1.0121x over previous
"""Supervised-contrastive loss on 8 Trainium2 NeuronCores — symmetric version.

Math (reference):
    z = x / max(||x||, 1e-8)                  row-normalize
    sim = (z @ z.T) / TEMP                    [N, N]
    per-anchor: pos-mean over same-class (excl. self) and logsumexp over
    j != i, then per-class mean, then mean over classes.

sim is symmetric, so only half of it is computed.  The matrix is tiled
into a 16x16 grid of [512, 512] blocks; block (a, b=(a+k) mod 16) for
k=0..8 covers every unordered pair exactly once (k=8 only for a<8).
Core c owns row-strips a=c (9 blocks) and a=c+8 (8 blocks) — 17 blocks
per core, identical program on every core (SPMD), only the data differs.
Each core's z8 columns are rolled by 512*c so its column ring is
memory-contiguous: strip 1 reads local cols [0, 4608), strip 2 reads
[4096, 8192).

Per block the core computes sim rows (fp8 DoubleRow matmuls, fp32 PSUM),
then ScalarE evaluates e = exp(10*sim - 2) writing fp8 to SBUF with the
row-sums riding accum_out.  For off-diagonal blocks the column-sums
(= the transposed rows' exp sums) are recovered with a ones-stationary
DoubleRow matmul over the fp8 exp values — 1/24 of the main matmul cost.
Class-segment sums tm = A @ W.T (W[c] = class-summed z) run at the END
so the tensor engine has scalar-independent work under the final ACT
drain.  Host assembles es[i] from row + column partials, subtracts the
exact diagonal exp(10*||z8_i||^2), and finishes the scalar loss.

Startup is DMA-trigger-bound (~620ns per trigger after a ~7us fixed
engine-init preamble), so the first chunk is only 512 wide and the DMA
order is: a8[m0] (96KB), the three 512-col z8 tiles it needs, then the
rest in consumption order.

Hardware pitfalls baked in (from the baseline's crash log): DMAs only
from nc.sync, one matmul accumulation group per PSUM bank, DoubleRow
outputs only with all 128 partitions, one EXP per psum tile.
"""

import numpy as np
import ml_dtypes

N = 8192           # anchors
D = 768            # feature dim
NOP = 64           # number of classes
CORES = 8
G = 512            # row/col block size
NGRP = N // G      # 16 groups
KT8 = D // 256     # 3 double-row contraction tiles
GW = 2048          # z8 DMA chunk width
NG = N // GW       # 4 DMA groups
TEMP_INV = 10.0
EPS = 1e-8
EBIAS = -2.0       # exp(10*sim + EBIAS); undone on host

# (strip, chunk) list: (col base, width); strip1 = blocks k=0..8 at local
# cols [0, 4608), strip2 = k=0..7 at [4096, 8192).
CHUNKS = [
    (0, 0, 512), (0, 512, 1536), (0, 2048, 1536), (0, 3584, 1024),
    (1, 4096, 1536), (1, 5632, 1536), (1, 7168, 1024),
]
NSLOT = 4 * len(CHUNKS)   # 28 pacc slots: chunk_lin*4 + m
NCS = 15                  # exported column-sum vectors per core

FP8 = ml_dtypes.float8_e4m3

_CACHE = {}
LAST_RESULT = None  # BassKernelResults of the most recent run (for profiling)


def _cs_blocks():
    """Host/device-shared program-order list of colsum exports.

    Yields (strip s, k distance) for every off-diagonal block.
    """
    out = []
    for s, base, width in CHUNKS:
        sbase = 0 if s == 0 else 4096
        for j in range(width // G):
            k = (base - sbase) // G + j
            if k == 0:
                continue  # diagonal block: no colsum
            out.append((s, k))
    return out


def _build_nc():
    from concourse import bacc
    import concourse.mybir as mybir
    import concourse.tile as tile

    f8 = mybir.dt.float8e4
    f32 = mybir.dt.float32
    Exp = mybir.ActivationFunctionType.Exp
    DR = mybir.MatmulPerfMode.DoubleRow

    nc = bacc.Bacc(
        "TRN2", target_bir_lowering=False, debug=False, enable_asserts=False
    )
    z8 = nc.dram_tensor("z8", [128, KT8, NG, 2, GW], f8, kind="ExternalInput").ap()
    a8 = nc.dram_tensor("a8", [128, 8, KT8, 2, 128], f8, kind="ExternalInput").ap()
    w8 = nc.dram_tensor("w8", [128, KT8, 2, NOP], f8, kind="ExternalInput").ap()
    on8 = nc.dram_tensor("on8", [128, 2, 128], f8, kind="ExternalInput").ap()
    pout = nc.dram_tensor("pout", [128, NSLOT], f32, kind="ExternalOutput").ap()
    cs = nc.dram_tensor("cs", [NCS, G], f32, kind="ExternalOutput").ap()
    tm = nc.dram_tensor("tm", [NOP, 2, 512], f32, kind="ExternalOutput").ap()

    with tile.TileContext(nc) as tc:
        with (
            tc.tile_pool(name="zin", bufs=KT8 * (NG + 1)) as zin,
            tc.tile_pool(name="expp", bufs=4) as expp,
            tc.tile_pool(name="csb", bufs=2) as csb,
            tc.tile_pool(name="singles", bufs=1) as singles,
        ):
            # ---- input DMAs, in consumption order ----
            a8_sb = singles.tile([128, 8, KT8, 2, 128], f8)
            nc.sync.dma_start(out=a8_sb[:, 0], in_=a8[:, 0])

            z8_sb = {}  # (g, kk) -> (tile, col offset within tile)

            def dma_z8(g, kk, lo, hi):
                z8_t = zin.tile([128, 2, hi - lo], f8, name="z8_t", tag="z8_t")
                nc.sync.dma_start(out=z8_t, in_=z8[:, kk, g, :, lo:hi])
                return z8_t

            # first 512-col block of group 0, then the rest of group 0
            z8_first = [dma_z8(0, kk, 0, G) for kk in range(KT8)]
            nc.sync.dma_start(out=a8_sb[:, 1:4], in_=a8[:, 1:4])
            nc.sync.dma_start(out=a8_sb[:, 4:8], in_=a8[:, 4:8])
            on8_sb = singles.tile([128, 2, 128], f8)
            nc.sync.dma_start(out=on8_sb, in_=on8)
            z8_rest = [dma_z8(0, kk, G, GW) for kk in range(KT8)]
            w8_sb = singles.tile([128, KT8, 2, NOP], f8)
            nc.sync.dma_start(out=w8_sb, in_=w8)
            for g in range(1, NG):
                for kk in range(KT8):
                    z8_sb[(g, kk)] = (dma_z8(g, kk, 0, GW), 0)

            def z8_slice(col, kk):
                """[128, 2, 512] moving operand for local col block."""
                g, off = divmod(col, GW)
                if g == 0:
                    if off == 0:
                        return z8_first[kk]
                    return z8_rest[kk][:, :, off - G:off - G + G]
                t, _ = z8_sb[(g, kk)]
                return t[:, :, off:off + G]

            pacc = singles.tile([128, NSLOT], f32)
            ebias_sb = singles.tile([128, 1], f32)
            nc.vector.memset(ebias_sb, EBIAS)

            ps_pool = tc.alloc_tile_pool(name="ps", bufs=2, space="PSUM")
            cs_pool = tc.alloc_tile_pool(name="csp", bufs=2, space="PSUM")

            def emit_tm():
                # tm.T = W8 @ A.T as two [64, 512] outputs: 12 wide matmuls
                # that slot under the final chunk's ACT drain.
                tm_sb = singles.tile([NOP, 2, 512], f32)
                for half in range(2):
                    pst = ps_pool.tile([NOP, 512], f32, name="ps_t", tag="ps_t")
                    first, last = (0, 0), (KT8 - 1, 1)
                    for kk in range(KT8):
                        for pl in range(2):
                            nc.tensor.matmul(
                                pst,
                                w8_sb[:, kk, pl, :],
                                a8_sb[:, half * 4:half * 4 + 4, kk, pl, :],
                                start=((kk, pl) == first),
                                stop=((kk, pl) == last),
                            )
                    nc.vector.tensor_copy(tm_sb[:, half, :], pst)
                nc.sync.dma_start(out=tm, in_=tm_sb)

            # ---- main symmetric slab ----
            cs_idx = 0
            for ch, (s, base, width) in enumerate(CHUNKS):
                nblk = width // G
                sbase = 0 if s == 0 else 4096
                exp_t = [
                    expp.tile([128, 2, width], f8, name="exp_t", tag="exp_t")
                    for _ in range(2)
                ]
                for m in range(4):
                    ps_t = ps_pool.tile([128, width], f32, name="ps_t", tag="ps_t")
                    for kk in range(KT8):
                        lhsT = a8_sb[:, s * 4 + m, kk, :, :]
                        for j in range(nblk):
                            nc.tensor.matmul(
                                ps_t[:, j * G:(j + 1) * G],
                                lhsT,
                                z8_slice(base + j * G, kk),
                                start=(kk == 0),
                                stop=(kk == KT8 - 1),
                                perf_mode=DR,
                            )
                    pair, plane = divmod(m, 2)
                    nc.scalar.activation(
                        out=exp_t[pair][:, plane, :],
                        in_=ps_t,
                        func=Exp,
                        scale=TEMP_INV,
                        bias=ebias_sb,
                        accum_out=pacc[:, ch * 4 + m:ch * 4 + m + 1],
                    )
                if ch == len(CHUNKS) - 1:
                    # tensor-only work in-stream BEFORE the cs matmuls that
                    # wait on the last ACTs — fills the drain window
                    emit_tm()
                # column sums of the off-diagonal blocks in this chunk
                for j in range(nblk):
                    if base + j * G == sbase:
                        continue  # diagonal block
                    csp = cs_pool.tile([128, G], f32, name="cs_t", tag="cs_t")
                    for pair in range(2):
                        nc.tensor.matmul(
                            csp,
                            on8_sb,
                            exp_t[pair][:, :, j * G:(j + 1) * G],
                            start=(pair == 0),
                            stop=(pair == 1),
                            perf_mode=DR,
                        )
                    cst = csb.tile([128, G], f32, name="cs_sb", tag="cs_sb")
                    nc.vector.tensor_copy(cst, csp)
                    nc.sync.dma_start(
                        out=cs[cs_idx:cs_idx + 1, :], in_=cst[0:1, :]
                    )
                    cs_idx += 1
            assert cs_idx == NCS

            nc.sync.dma_start(out=pout, in_=pacc)

            cs_pool.release()
            ps_pool.release()

    nc.compile()
    return nc


def _get_nc():
    if "nc" not in _CACHE:
        _CACHE["nc"] = _build_nc()
    return _CACHE["nc"]


def _pack_dr(mat_t):
    """[D, cols] -> [128, KT8, 2, cols] with d = kk*256 + i*128 + p."""
    d, cols = mat_t.shape
    return np.ascontiguousarray(
        mat_t.reshape(KT8, 2, 128, cols).transpose(2, 0, 1, 3)
    )


def kernel(x, op_ids, n_op):
    global LAST_RESULT
    from concourse.bass_utils import run_bass_kernel_spmd

    x = np.asarray(x, dtype=np.float32).reshape(-1, D)
    op_ids = np.asarray(op_ids).reshape(-1).astype(np.int64)
    n_op_i = int(np.asarray(n_op))

    # ---- host prep: normalize, quantize, class sums, diagonal ----
    norms = np.sqrt((x.astype(np.float64) ** 2).sum(axis=1))
    norms = np.maximum(norms, EPS).astype(np.float32)
    z = x / norms[:, None]

    z8 = z.astype(FP8)
    z8f = z8.astype(np.float32)

    onehot = np.zeros((N, NOP), np.float32)
    onehot[np.arange(N), op_ids] = 1.0
    W8 = (onehot.T @ z8f).astype(FP8)               # [NOP, D] fp8

    z8_packed = _pack_dr(np.ascontiguousarray(z8.T))          # [128,3,2,N]
    w8_packed = _pack_dr(np.ascontiguousarray(W8.T.astype(FP8)))
    ones8 = np.ones((128, 2, 128), FP8)
    ssq = (z8f.astype(np.float64) ** 2).sum(axis=1)  # = sim[i, i]

    in_maps = []
    for c in range(CORES):
        # rolled columns: local col t = global (512c + t) mod N
        zc = np.roll(z8_packed, -G * c, axis=3)
        zc_chunked = np.ascontiguousarray(
            zc.reshape(128, KT8, 2, NG, GW).transpose(0, 1, 3, 2, 4)
        )
        rows = np.concatenate(
            [
                z8_packed[:, :, :, c * G:(c + 1) * G],
                z8_packed[:, :, :, (c + 8) * G:(c + 9) * G],
            ],
            axis=3,
        )  # [128, 3, 2, 1024]
        a8_m = np.ascontiguousarray(
            rows.reshape(128, KT8, 2, 8, 128).transpose(0, 3, 1, 2, 4)
        )  # [128, 8, 3, 2, 128]
        in_maps.append(
            {
                "z8": zc_chunked,
                "a8": a8_m,
                "w8": w8_packed,
                "on8": ones8,
            }
        )

    nc = _get_nc()
    res = run_bass_kernel_spmd(nc, in_maps, core_ids=list(range(CORES)))
    LAST_RESULT = res

    # ---- host post: assemble es from row+col partials, finish loss ----
    cs_order = _cs_blocks()
    es_scaled = np.zeros(N, np.float64)
    tm_full = np.empty((N, NOP), np.float64)
    n_ch1 = sum(1 for s, _, _ in CHUNKS if s == 0)
    for c in range(CORES):
        pout_c = res.results[c]["pout"].astype(np.float64)  # [128, NSLOT]
        cs_c = res.results[c]["cs"].astype(np.float64)      # [NCS, G]
        tm_c = res.results[c]["tm"].astype(np.float64)      # [NOP, 2, 512]
        for s in range(2):
            a = c + 8 * s
            rows = slice(a * G, (a + 1) * G)
            lo = 0 if s == 0 else n_ch1
            hi = n_ch1 if s == 0 else len(CHUNKS)
            slots = pout_c[:, lo * 4:hi * 4].reshape(128, hi - lo, 4)
            es_scaled[rows] += slots.sum(axis=1).T.reshape(G)
            tm_full[rows] = tm_c[:, s, :].T
        for i, (s, k) in enumerate(cs_order):
            b = (c + 8 * s + k) % NGRP
            es_scaled[b * G:(b + 1) * G] += cs_c[i]

    es = es_scaled * np.exp(-EBIAS)
    lse = np.log(es - np.exp(TEMP_INV * ssq))
    pos_sum = TEMP_INV * (tm_full[np.arange(N), op_ids] - ssq)
    counts = np.bincount(op_ids, minlength=n_op_i).astype(np.float64)
    pos_cnt = counts[op_ids] - 1.0

    loss_i = np.where(pos_cnt > 0, -pos_sum / np.maximum(pos_cnt, 1.0) + lse, 0.0)
    cls_sum = np.bincount(op_ids, weights=loss_i, minlength=n_op_i)
    cls_loss = np.where(counts > 0, cls_sum / np.maximum(counts, 1.0), 0.0)
    return np.float32(cls_loss.mean())
